# revision 21
# baseline (speedup 1.0000x reference)
"""DropBlock kernel for Trainium2, 8 NeuronCores, batch-sharded data parallel.

Reference computation (B,C,H,W = 128,64,56,56, block=5, gamma=0.02):
    mask    = (noise < gamma)                       # (B,C,52,52) corner drops
    dilated = maxpool5x5_full_pad(mask)             # (B,C,56,56)
    block_mask = 1 - dilated
    out = block_mask * x * (numel / sum(block_mask))

Kernel formulation (exact, no mask materialization in f32):
    d = noise - gamma  (f32 subtract; sign/zero exact by Sterbenz, then bf16
                        cast which preserves sign and never rounds to 0)
    block_mask[h,w] = ( min_{5x5 window}(d) >= 0 )  # min-pool == dilated drop
    count = sum(block_mask) via fused accum, AllReduce across 8 cores.

Each core: 16 batches x 64 ch = 1024 images -> 8 tiles of 128 images
(images on partitions, image pixels along the free dimension).  The 5-wide
separable min uses log-step shifts (3 tensor_tensor ops per axis) on
1.0-padded buffers so no boundary special cases are needed.
"""

import sys

sys.path.insert(0, "/opt/trn_rl_repo")

import numpy as np

import concourse.bacc as bacc
import concourse.bass as bass
import concourse.tile as tile
import concourse.mybir as mybir
from concourse import bass_isa
from concourse.bass_utils import run_bass_kernel_spmd

N_CORES = 8
B, C, H, W = 128, 64, 56, 56
BLK = 5
GAMMA = 0.02
NH, NW = H - (BLK - 1), W - (BLK - 1)  # 52, 52 noise dims
B_SH = B // N_CORES  # 16 batches per core
IMGS = B_SH * C  # 1024 images per core
P = 128  # partitions
NTILES = IMGS // P  # 8 tiles per core
NPIX = NH * NW  # 2704 noise pixels/image
OPIX = H * W  # 3136 out pixels/image
TROWS = NH + 2 * (BLK - 1)  # 60 rows in padded vertical buffer
TFLAT = TROWS * NW  # 3120
VPW = NW + 2 * (BLK - 1)  # 60 cols in padded horizontal buffer (4+52+4)
COUNT_M = float(B * C * H * W)  # 25690112.0

# Largest f32 strictly below 0.02f: keep ⟺ noise >= 0.02f ⟺ noise-γ' > 0,
# so the mask is Relu(Sign(min-pool(noise-γ'))) with exact {0,1} handling.
GAMMA_LO = float(np.nextafter(np.float32(GAMMA), np.float32(0)))

F32 = mybir.dt.float32
BF16 = mybir.dt.bfloat16
MIN = mybir.AluOpType.min
MULT = mybir.AluOpType.mult

_CACHE = {}


def _build(single_core=False):
    """Build + compile the SPMD bass module once.

    single_core=True builds a collective-free variant (the per-core count is
    used directly as the global count) for cost-model simulation only.
    """
    nc = bacc.Bacc("TRN2", target_bir_lowering=False, debug=False,
                   num_devices=1 if single_core else N_CORES)
    noise_ap = nc.dram_tensor("noise", [IMGS, NPIX], F32,
                              kind="ExternalInput").ap()
    x_ap = nc.dram_tensor("x", [IMGS, OPIX], F32, kind="ExternalInput").ap()
    out_ap = nc.dram_tensor("out", [IMGS, OPIX], F32,
                            kind="ExternalOutput").ap()

    with tile.TileContext(nc) as tc:
        with (
            tc.tile_pool(name="nraw", bufs=2) as nraw_pool,
            tc.tile_pool(name="tbuf", bufs=2) as t_pool,
            tc.tile_pool(name="wa", bufs=2) as a_pool,
            tc.tile_pool(name="wb", bufs=2) as b_pool,
            tc.tile_pool(name="vp", bufs=2) as vp_pool,
            tc.tile_pool(name="a2", bufs=2) as a2_pool,
            tc.tile_pool(name="b2", bufs=2) as b2_pool,
            tc.tile_pool(name="dmin", bufs=2) as dmin_pool,
            tc.tile_pool(name="mask", bufs=1) as mask_pool,
            tc.tile_pool(name="stats", bufs=1) as stats_pool,
            tc.tile_pool(name="xio", bufs=3) as x_pool,
            tc.tile_pool(name="dram", bufs=1, space="DRAM") as dram_pool,
        ):
            mask_store = mask_pool.tile([P, NTILES * OPIX], BF16)
            partials = stats_pool.tile([P, NTILES], F32)
            gbias = stats_pool.tile([P, 1], F32)
            nc.vector.memset(gbias[:], -GAMMA_LO)

            # ---------------- phase 1: block mask + counts ----------------
            X_PREFETCH = 3
            xts = {}
            for t in range(NTILES):
                if t < X_PREFETCH:
                    # prefetch x tiles early; the scalar-engine HWDGE queue
                    # has no collective-dependent work, so these overlap
                    # with phase 1.
                    xts[t] = x_pool.tile([P, OPIX], F32, name=f"xt{t}", tag="xt")
                    nc.scalar.dma_start(xts[t][:], x_ap[bass.ts(t, P), :])
                nraw = nraw_pool.tile([P, NPIX], F32)
                nc.sync.dma_start(nraw[:], noise_ap[bass.ts(t, P), :])

                # T: (60,52) bf16, rows 0..3 / 56..59 = 1.0 pad,
                # rows 4..55 = noise - gamma
                tb = t_pool.tile([P, TFLAT], BF16)
                nc.gpsimd.memset(tb[:, 0:(BLK - 1) * NW], 1.0)
                nc.gpsimd.memset(tb[:, (NH + BLK - 1) * NW:TFLAT], 1.0)
                nc.scalar.activation(
                    tb[:, (BLK - 1) * NW:(NH + BLK - 1) * NW], nraw[:],
                    mybir.ActivationFunctionType.Identity,
                    bias=gbias[:, 0:1])

                # vertical min pool, log-step: windows of 2, 4, then 5
                a = a_pool.tile([P, (TROWS - 1) * NW], BF16)  # 59 rows
                nc.vector.tensor_tensor(
                    a[:], tb[:, 0:(TROWS - 1) * NW], tb[:, NW:TFLAT], MIN)
                bt = b_pool.tile([P, (TROWS - 3) * NW], BF16)  # 57 rows
                nc.vector.tensor_tensor(
                    bt[:], a[:, 0:(TROWS - 3) * NW],
                    a[:, 2 * NW:(TROWS - 1) * NW], MIN)
                # V[r] = min(B[r], T[r+4]), r in 0..55 -> into padded Vp
                vp = vp_pool.tile([P, H * VPW], BF16)
                vp3 = vp[:].rearrange("p (h w) -> p h w", w=VPW)
                nc.gpsimd.memset(vp3[:, :, 0:BLK - 1], 1.0)
                nc.gpsimd.memset(vp3[:, :, W:VPW], 1.0)
                bt3 = bt[:].rearrange("p (h w) -> p h w", w=NW)
                tb3 = tb[:].rearrange("p (h w) -> p h w", w=NW)
                nc.vector.tensor_tensor(
                    vp3[:, :, BLK - 1:BLK - 1 + NW], bt3[:, 0:H, :],
                    tb3[:, BLK - 1:TROWS, :], MIN)

                # horizontal min pool, log-step (flat shifted APs; the
                # out-of-row tail elements are junk but never read)
                HV = H * VPW
                a2 = a2_pool.tile([P, HV], BF16)
                nc.vector.tensor_tensor(
                    a2[:, 0:HV - 1], vp[:, 0:HV - 1], vp[:, 1:HV], MIN)
                b2 = b2_pool.tile([P, HV], BF16)
                nc.vector.tensor_tensor(
                    b2[:, 0:HV - 2], a2[:, 0:HV - 2], a2[:, 2:HV], MIN)
                b23 = b2[:].rearrange("p (h w) -> p h w", w=VPW)
                dm = dmin_pool.tile([P, OPIX], BF16)
                dm3 = dm[:].rearrange("p (h w) -> p h w", w=W)
                nc.vector.tensor_tensor(
                    dm3[:, :, :], b23[:, :, 0:W], vp3[:, :, BLK - 1:VPW], MIN)

                # block_mask = (dmin > 0) = Relu(Sign(dmin)) on the scalar
                # engine (keeps DVE free); count via fused accum.
                nc.scalar.activation(dm[:], dm[:],
                                     mybir.ActivationFunctionType.Sign)
                nc.scalar.activation(
                    mask_store[:, t * OPIX:(t + 1) * OPIX], dm[:],
                    mybir.ActivationFunctionType.Relu,
                    accum_out=partials[:, t:t + 1])

            # ------------- global count -> scale = M / count_ones -------------
            ptot = stats_pool.tile([P, 1], F32)
            nc.vector.tensor_reduce(ptot[:], partials[:],
                                    mybir.AxisListType.X, mybir.AluOpType.add)
            pall = stats_pool.tile([P, 1], F32)
            nc.gpsimd.partition_all_reduce(pall[:], ptot[:], channels=P,
                                           reduce_op=bass_isa.ReduceOp.add)
            if single_core:
                tot_sb = pall
            else:
                cc_in = dram_pool.tile([P, 1], F32)
                cc_out = dram_pool.tile([P, 1], F32)
                nc.gpsimd.dma_start(cc_in[:], pall[:])
                nc.gpsimd.collective_compute(
                    "AllReduce", mybir.AluOpType.add,
                    replica_groups=[list(range(N_CORES))],
                    ins=[cc_in.opt()], outs=[cc_out.opt()])
                tot_sb = stats_pool.tile([P, 1], F32)
                nc.gpsimd.dma_start(tot_sb[:], cc_out[:])
            recip = stats_pool.tile([P, 1], F32)
            nc.vector.reciprocal(recip[:], tot_sb[:])
            scale_sb = stats_pool.tile([P, 1], F32)
            nc.vector.tensor_scalar_mul(scale_sb[:], recip[:], COUNT_M)

            # ---------------- phase 2: out = (x*scale)*mask ----------------
            for t in range(NTILES):
                if t in xts:
                    xt = xts[t]
                else:
                    xt = x_pool.tile([P, OPIX], F32, name=f"xt{t}", tag="xt")
                    nc.scalar.dma_start(xt[:], x_ap[bass.ts(t, P), :])
                nc.vector.scalar_tensor_tensor(
                    xt[:], xt[:], scale_sb[:, 0:1],
                    mask_store[:, t * OPIX:(t + 1) * OPIX], MULT, MULT)
                nc.sync.dma_start(out_ap[bass.ts(t, P), :], xt[:])

    nc.compile()
    return nc


def _get_nc():
    if "nc" not in _CACHE:
        _CACHE["nc"] = _build()
    return _CACHE["nc"]


def kernel(x: np.ndarray, noise: np.ndarray) -> np.ndarray:
    nc = _get_nc()
    in_maps = []
    for i in range(N_CORES):
        xs = np.ascontiguousarray(x[i * B_SH:(i + 1) * B_SH]).reshape(
            IMGS, OPIX)
        ns = np.ascontiguousarray(noise[i * B_SH:(i + 1) * B_SH]).reshape(
            IMGS, NPIX)
        in_maps.append({"x": xs, "noise": ns})
    res = run_bass_kernel_spmd(nc, in_maps, list(range(N_CORES)))
    out = np.empty((B, C, H, W), dtype=np.float32)
    for i in range(N_CORES):
        out[i * B_SH:(i + 1) * B_SH] = res.results[i]["out"].reshape(
            B_SH, C, H, W)
    return out
